# revision 23
# baseline (speedup 1.0000x reference)
# EEGGraphConvNet Trainium2 Bass kernel.
#
# Problem structure (hardcoded from the reference):
#   50000 graphs x 8 nodes, fully-connected 64 edges/graph (meshgrid order).
#   4 GCN layers 6->16->32->64->50 (lrelu on first 3), global BatchNorm(50),
#   lrelu, per-graph add-pool, MLP 50->30->20->2.
#
# Sharding: data-parallel over graphs, 6250 graphs/core on 8 cores.
# Per-core layout: blocks of 16 graphs = 128 nodes on the 128 partitions.
#   Aggregation per block: out^T = H^T @ blockdiag(A_g), computed on the PE
#   with the node-feature block as the (cheap, F-column) stationary operand
#   and the block-diagonal adjacency as the moving operand. blockdiag(A) is
#   built on-chip by a DVE broadcast-multiply against a 0/1 mask.
#   Transforms run channels-on-partitions with the shared W as stationary.
#   PE transpose flips layouts where structurally required.
# BatchNorm stats: bn_stats/bn_aggr per core + AllReduce over 8 cores.

import os
import sys

import numpy as np

if not any(os.path.isdir(os.path.join(p, "concourse")) for p in sys.path if p):
    sys.path.insert(0, "/opt/trn_rl_repo")

import concourse.bass as bass
import concourse.bacc as bacc
import concourse.mybir as mybir
import concourse.tile as tile
from concourse.bass_utils import run_bass_kernel_spmd

F32 = mybir.dt.float32
AF = mybir.ActivationFunctionType
OP = mybir.AluOpType

NPG = 8
EPG = 64
NUM_GRAPHS = 50000
N_NODES = NUM_GRAPHS * NPG
NCORES = 8
GPC = NUM_GRAPHS // NCORES          # 6250 graphs per core
BG = 16                             # graphs per 128-partition block
EPS = 1e-5
SLOPE = 0.01

_cache = {}
TRACE = False          # set True (e.g. from test.py) to capture an NTFF profile
LAST_RESULTS = None    # BassKernelResults of the most recent run


def _block_counts(gpc):
    nb = (gpc + BG - 1) // BG       # blocks per core
    gpad = nb * BG
    nbh = (nb + 1) // 2             # even-half block slots
    return nb, gpad, nbh



def _emit_affine_lrelu(nc, out, in_, bias=0.0, scale=1.0, lrelu=True):
    """out = lrelu(in_*scale + bias); Identity on ACT + max(x, s*x) on DVE."""
    nc.scalar.activation(out, in_, AF.Identity, bias=bias, scale=scale)
    if lrelu:
        nc.vector.scalar_tensor_tensor(out=out, in0=out, scalar=SLOPE, in1=out,
                                       op0=OP.mult, op1=OP.max)


def build_program(gpc=GPC, n_total=N_NODES):
    """Build the SPMD Bass program for one core (all cores identical)."""
    nb, gpad, nbh = _block_counts(gpc)
    gcols = nbh * BG * 2            # pooled/logit columns (even half | odd half)

    nc = bacc.Bacc(None, num_devices=NCORES)

    # ---- external I/O ----------------------------------------------------
    xin = nc.declare_dram_parameter("xin", [128, nb * 6], F32, isOutput=False)
    ewin = nc.declare_dram_parameter("ewin", [128, nb * 8], F32, isOutput=False)
    wshapes = dict(w1=(6, 16), w2=(16, 32), w3=(32, 64), w4=(64, 50),
                   wf1=(50, 30), wf2=(30, 20), wf3=(20, 2))
    wp = {k: nc.declare_dram_parameter(k, list(s), F32, isOutput=False)
          for k, s in wshapes.items()}
    bshapes = dict(b1=16, b2=32, b3=64, bf1=30, bf2=20, bf3=2)
    bp = {k: nc.declare_dram_parameter(k, [n, 1], F32, isOutput=False)
          for k, n in bshapes.items()}
    # stacked [x; x] for the two-half h4 layout
    b4s = nc.declare_dram_parameter("b4s", [128, 1], F32, isOutput=False)
    gammas = nc.declare_dram_parameter("gammas", [128, 1], F32, isOutput=False)
    betas = nc.declare_dram_parameter("betas", [128, 1], F32, isOutput=False)
    outp = nc.declare_dram_parameter("outp", [2, gcols], F32, isOutput=True)

    # ---- constants -------------------------------------------------------
    maskd = np.zeros((128, 128), np.float32)
    for p in range(128):
        g = p // 8
        maskd[p, g * 8:(g + 1) * 8] = 1.0
    ident = np.eye(128, dtype=np.float32)
    # pad mask for the last block: zero columns of padded graphs
    gpad_last = gpc - (nb - 1) * BG   # real graphs in last block
    pmaskd = np.ones((50, 128), np.float32)
    pmaskd[:, gpad_last * 8:] = 0.0
    mask_d = nc.inline_tensor(maskd, "maskc")
    ident_d = nc.inline_tensor(ident, "identc")
    pmask_d = nc.inline_tensor(pmaskd, "pmaskc")

    layer_cfg = [  # (W, b, Fi, Fo) for layers 1..3 (agg-first, lrelu)
        ("w1", "b1", 6, 16),
        ("w2", "b2", 16, 32),
        ("w3", "b3", 32, 64),
    ]

    with tile.TileContext(nc) as tc:
        with (
            tc.tile_pool(name="big", bufs=1) as big,
            tc.tile_pool(name="wts", bufs=1) as wts,
            tc.tile_pool(name="blk", bufs=8) as blkp,
            tc.tile_pool(name="work", bufs=3) as work,
            tc.tile_pool(name="hwork", bufs=3) as hwork,
            tc.tile_pool(name="stat", bufs=1) as statp,
            tc.tile_pool(name="ps", bufs=2, space="PSUM") as ps,
            tc.tile_pool(name="psT", bufs=2, space="PSUM") as psT,
            tc.tile_pool(name="dram", bufs=1, space="DRAM") as dramp,
        ):
            # persistent SBUF buffers
            X = big.tile([128, nb, 6], F32, tag="X")
            H4 = big.tile([128, nbh, 128], F32, tag="H4")
            MASKD = wts.tile([128, 16, 8], F32, tag="MASKD")
            MASK = wts.tile([128, 16, 8], F32, tag="MASK")
            IDENT = wts.tile([128, 128], F32, tag="IDENT")
            PMASK = wts.tile([50, 128], F32, tag="PMASK")

            nc.sync.dma_start(out=X[:], in_=xin[:].rearrange("p (b c) -> p b c", c=6))
            nc.sync.dma_start(out=MASKD[:], in_=mask_d[:].rearrange("p (b c) -> p b c", c=8))
            # DVE-local copy: blkA builds then need no MASK-DMA wait (FIFO)
            nc.vector.tensor_copy(MASK[:], MASKD[:])
            nc.sync.dma_start(out=IDENT[:], in_=ident_d[:])
            nc.sync.dma_start(out=PMASK[:], in_=pmask_d[:])

            W = {}
            for k, s in wshapes.items():
                W[k] = wts.tile(list(s), F32, tag=k, name=f"W_{k}")
                nc.sync.dma_start(out=W[k][:], in_=wp[k][:])
            B = {}
            for k, n in bshapes.items():
                B[k] = wts.tile([n, 1], F32, tag=k, name=f"B_{k}")
                nc.sync.dma_start(out=B[k][:], in_=bp[k][:])
            B4S = wts.tile([128, 1], F32, tag="b4s")
            GAM = wts.tile([128, 1], F32, tag="gammas")
            BET = wts.tile([128, 1], F32, tag="betas")
            nc.sync.dma_start(out=B4S[:], in_=b4s[:])
            nc.sync.dma_start(out=GAM[:], in_=gammas[:])
            nc.sync.dma_start(out=BET[:], in_=betas[:])

            # zero unused rows (50:64, 114:128) and the unused odd slot
            nc.vector.memset(H4[32:64, :, :], 0.0)
            nc.vector.memset(H4[96:128, :, :], 0.0)
            nodd = nb // 2
            if nodd < nbh:
                nc.vector.memset(H4[64:96, nodd:nbh, :], 0.0)

            # ---- phase A: per-block GCN stack ---------------------------
            for b in range(nb):
                ewb = blkp.tile([128, 8], F32, tag="ewb")
                nc.sync.dma_start(out=ewb[:], in_=ewin[:, b * 8:(b + 1) * 8])
                blkA = blkp.tile([128, 128], F32, tag="blkA")
                nc.vector.tensor_tensor(
                    out=blkA[:].rearrange("p (g j) -> p g j", j=8),
                    in0=ewb[:, None, :].broadcast_to((128, 16, 8)),
                    in1=MASK[:],
                    op=OP.mult,
                )
                # layers 1..3: Z^T = H^T blkA ; H' = lrelu(W^T Z^T + b); flip
                Hcur = X[:, b, :]          # [128, Fi] nodes-on-partitions
                HT = None
                for (wk, bk, Fi, Fo) in layer_cfg:
                    ZT_ps = ps.tile([64, 128], F32, tag="zt")
                    nc.tensor.matmul(ZT_ps[:Fi, :], lhsT=Hcur, rhs=blkA[:],
                                     start=True, stop=True)
                    ZT = work.tile([64, 128], F32, tag="zt_s")
                    nc.scalar.activation(ZT[:Fi, :], ZT_ps[:Fi, :], AF.Copy)
                    HT_ps = ps.tile([64, 128], F32, tag="ht")
                    nc.tensor.matmul(HT_ps[:Fo, :], lhsT=W[wk][:], rhs=ZT[:Fi, :],
                                     start=True, stop=True)
                    HT = work.tile([64, 128], F32, tag="ht_s")
                    _emit_affine_lrelu(nc, HT[:Fo, :], HT_ps[:Fo, :], bias=B[bk][:])
                    if Fo < 64 or True:
                        pass
                    # flip to nodes-on-partitions for the next aggregation;
                    # layer 3's flip is skipped (layer 4 consumes HT directly)
                    if wk != "w3":
                        Hn_ps = psT.tile([128, 64], F32, tag="hn")
                        nc.tensor.transpose(Hn_ps[:, :Fo], HT[:Fo, :],
                                            IDENT[:Fo, :Fo])
                        Hn = hwork.tile([128, 64], F32, tag="hn_s")
                        nc.vector.tensor_copy(Hn[:, :Fo], Hn_ps[:, :Fo])
                        Hcur = Hn[:, :Fo]
                # layer 4: transform-first (64->50), flip, aggregate, + b4
                U4T_ps = ps.tile([64, 128], F32, tag="ht")
                nc.tensor.matmul(U4T_ps[:50, :], lhsT=W["w4"][:], rhs=HT[:64, :],
                                 start=True, stop=True)
                U4T = work.tile([64, 128], F32, tag="ht_s")
                nc.scalar.activation(U4T[:50, :], U4T_ps[:50, :], AF.Copy)
                U4_ps = psT.tile([128, 64], F32, tag="hn")
                nc.tensor.transpose(U4_ps[:, :50], U4T[:50, :], IDENT[:50, :50])
                U4 = hwork.tile([128, 64], F32, tag="hn_s")
                nc.vector.tensor_copy(U4[:, :50], U4_ps[:, :50])
                half = 64 * (b % 2)
                Z4_ps = psT.tile([128, 128], F32, tag="z4")
                nc.tensor.matmul(Z4_ps[half:half + 50, :], lhsT=U4[:, :50],
                                 rhs=blkA[:], start=True, stop=True,
                                 tile_position=(0, half))
                dst = H4[half:half + 50, b // 2, :]
                nc.scalar.activation(dst, Z4_ps[half:half + 50, :], AF.Identity,
                                     bias=B4S[half:half + 50, :])
                if b == nb - 1:
                    nc.vector.tensor_tensor(out=dst, in0=dst, in1=PMASK[:],
                                            op=OP.mult)

            if os.environ.get("K_DEBUG_H4"):
                h4dbg = nc.declare_dram_parameter(
                    "h4dbg", [128, nbh * 128], F32, isOutput=True)
                nc.sync.dma_start(
                    out=h4dbg[:],
                    in_=H4[:].rearrange("p a b -> p (a b)"))
            # ---- phase B: BN stats + collective -------------------------
            sums = statp.tile([128, 2], F32, tag="sums")
            # sums[:,0] = sum(h4) ; sums[:,1] = sum(h4^2) over this core
            nc.vector.reduce_sum(sums[:, 0:1],
                                 H4[:].rearrange("p a b -> p (a b)"),
                                 axis=mybir.AxisListType.X)
            nchunk = min(16, nbh)
            bounds = [round(i * nbh / nchunk) for i in range(nchunk + 1)]
            sqp = statp.tile([128, nchunk], F32, tag="sqp")
            for ci in range(nchunk):
                s0, s1 = bounds[ci], bounds[ci + 1]
                scratch = work.tile([128, 1664], F32, tag="sqs",
                                    name="sq_scratch", bufs=1)
                width = (s1 - s0) * 128
                nc.scalar.activation(scratch[:, :width],
                                     H4[:, s0:s1, :].rearrange("p a b -> p (a b)"),
                                     AF.Square)
                nc.vector.reduce_sum(sqp[:, ci:ci + 1], scratch[:, :width],
                                     axis=mybir.AxisListType.X)
            nc.vector.reduce_sum(sums[:, 1:2], sqp[:], axis=mybir.AxisListType.X)

            cin = dramp.tile([128, 2], F32, tag="ccin")
            cout = dramp.tile([128, 2], F32, tag="ccout")
            nc.sync.dma_start(out=cin[:], in_=sums[:])
            nc.gpsimd.collective_compute(
                "AllReduce", OP.add,
                replica_groups=[list(range(NCORES))],
                ins=[cin[:]], outs=[cout[:]],
            )
            gsum = statp.tile([128, 2], F32, tag="gsum")
            nc.sync.dma_start(out=gsum[:], in_=cout[:])
            # fold halves: tot[c] = gsum[c] + gsum[c+50], replicated to both
            gsum2 = statp.tile([50, 2], F32, tag="gsum2")
            nc.sync.dma_start(out=gsum2[:], in_=gsum[64:114, :])
            tot = statp.tile([128, 2], F32, tag="tot")
            nc.vector.memset(tot[:], 0.0)
            nc.vector.tensor_tensor(out=tot[0:50, :], in0=gsum[0:50, :],
                                    in1=gsum2[:], op=OP.add)
            nc.sync.dma_start(out=tot[64:114, :], in_=tot[0:50, :])
            # mu = tot0/N ; var = tot1/N - mu^2 ; s = gamma/sqrt(var+eps)
            # t = beta - mu*s
            ninv = 1.0 / float(n_total)
            mu = statp.tile([128, 1], F32, tag="mu")
            var = statp.tile([128, 1], F32, tag="var")
            nc.vector.tensor_scalar(out=mu[:], in0=tot[:, 0:1],
                                    scalar1=ninv, scalar2=None, op0=OP.mult)
            musq = statp.tile([128, 1], F32, tag="musq")
            nc.vector.tensor_tensor(out=musq[:], in0=mu[:], in1=mu[:], op=OP.mult)
            nc.vector.tensor_scalar(out=var[:], in0=tot[:, 1:2],
                                    scalar1=ninv, scalar2=None, op0=OP.mult)
            nc.vector.tensor_tensor(out=var[:], in0=var[:], in1=musq[:],
                                    op=OP.subtract)
            if os.environ.get("K_DEBUG_STATS"):
                sdbg = nc.declare_dram_parameter("sdbg", [128, 8], F32, isOutput=True)
                nc.sync.dma_start(out=sdbg[:, 2:4], in_=sums[:])
                nc.sync.dma_start(out=sdbg[:, 4:6], in_=tot[:])
                nc.sync.dma_start(out=sdbg[:, 6:7], in_=mu[:])
                nc.sync.dma_start(out=sdbg[:, 7:8], in_=var[:])
            rs = statp.tile([128, 1], F32, tag="rs")
            nc.vector.tensor_scalar(out=rs[:], in0=var[:], scalar1=EPS,
                                    scalar2=None, op0=OP.add)
            nc.vector.reciprocal(rs[:], rs[:])
            nc.scalar.sqrt(rs[:], rs[:])            # rs = 1/sqrt(var+eps)
            sco = statp.tile([128, 1], F32, tag="sco")
            tco = statp.tile([128, 1], F32, tag="tco")
            nc.vector.tensor_tensor(out=sco[:], in0=GAM[:], in1=rs[:], op=OP.mult)
            nc.vector.tensor_tensor(out=tco[:], in0=mu[:], in1=sco[:], op=OP.mult)
            nc.vector.tensor_tensor(out=tco[:], in0=BET[:], in1=tco[:],
                                    op=OP.subtract)


            # ---- phase C: BN+lrelu, pool, MLP ---------------------------
            _emit_affine_lrelu(nc, H4[:], H4[:], bias=tco[:], scale=sco[:])
            POOL = big.tile([128, nbh * 16], F32, tag="POOL")
            nc.vector.reduce_sum(
                POOL[:].rearrange("p (b g) -> p b g", g=16),
                H4[:].rearrange("p b (g e) -> p b g e", e=8),
                axis=mybir.AxisListType.X,
            )
            PALL = big.tile([50, 2 * nbh * 16], F32, tag="H4")
            half_cols = nbh * 16
            nc.vector.tensor_copy(PALL[:, 0:half_cols], POOL[0:50, :])
            nc.sync.dma_start(out=PALL[:, half_cols:], in_=POOL[64:114, :])

            LOG = big.tile([2, gcols], F32, tag="LOG")
            CH = 512
            for c0 in range(0, gcols, CH):
                c1 = min(c0 + CH, gcols)
                m1 = ps.tile([30, CH], F32, tag="zt")
                nc.tensor.matmul(m1[:, :c1 - c0], lhsT=W["wf1"][:],
                                 rhs=PALL[:, c0:c1], start=True, stop=True)
                a1 = work.tile([30, CH], F32, tag="a1", bufs=2)
                _emit_affine_lrelu(nc, a1[:, :c1 - c0], m1[:, :c1 - c0], bias=B["bf1"][:])
                m2 = ps.tile([20, CH], F32, tag="ht")
                nc.tensor.matmul(m2[:, :c1 - c0], lhsT=W["wf2"][:],
                                 rhs=a1[:, :c1 - c0], start=True, stop=True)
                a2 = work.tile([20, CH], F32, tag="a2", bufs=2)
                _emit_affine_lrelu(nc, a2[:, :c1 - c0], m2[:, :c1 - c0], bias=B["bf2"][:])
                m3 = psT.tile([2, CH], F32, tag="hn")
                nc.tensor.matmul(m3[:, :c1 - c0], lhsT=W["wf3"][:],
                                 rhs=a2[:, :c1 - c0], start=True, stop=True)
                nc.scalar.activation(LOG[:, c0:c1], m3[:, :c1 - c0], AF.Identity,
                                     bias=B["bf3"][:])

            nc.sync.dma_start(out=outp[:], in_=LOG[:])

    nc.compile()
    return nc, nb, gpad, nbh, gcols


def _prep_core_inputs(xs, ews, nb, gpad):
    """Relayout one core's x [gpc*8, 6] and A [gpc, 8, 8] to SBUF order."""
    gpc = ews.shape[0]
    xp = np.zeros((gpad * NPG, 6), np.float32)
    xp[: gpc * NPG] = xs
    ep = np.zeros((gpad, 8, 8), np.float32)
    ep[:gpc] = ews
    # [nb, 16, 8, c] -> partition (gl*8+i), free (b, c)
    xin = xp.reshape(nb, BG, NPG, 6).transpose(1, 2, 0, 3).reshape(128, nb * 6)
    ewin = ep.reshape(nb, BG, NPG, 8).transpose(1, 2, 0, 3).reshape(128, nb * 8)
    return np.ascontiguousarray(xin), np.ascontiguousarray(ewin)


def _edge_matrices(edge_index, edge_weight):
    """Per-graph dense A[g, i, j] = sum of ew over edges (g,i)->(g,j)."""
    src = np.asarray(edge_index[0])
    dst = np.asarray(edge_index[1])
    ew = np.asarray(edge_weight, np.float32)
    gs, gi = np.divmod(src, NPG)
    gd, gj = np.divmod(dst, NPG)
    # fast path: the canonical meshgrid layout of the reference
    e = np.arange(src.shape[0])
    if (
        src.shape[0] == NUM_GRAPHS * EPG
        and np.array_equal(gs, e // EPG)
        and np.array_equal(gd, e // EPG)
        and np.array_equal(gi, (e % EPG) // NPG)
        and np.array_equal(gj, e % NPG)
    ):
        return ew.reshape(NUM_GRAPHS, 8, 8)
    if not np.array_equal(gs, gd):
        raise ValueError("cross-graph edges are not supported by this kernel")
    A = np.zeros((NUM_GRAPHS, 8, 8), np.float32)
    np.add.at(A, (gs, gi, gj), ew)
    return A


def _stack2(v):
    out = np.zeros((128, 1), np.float32)
    v = np.asarray(v, np.float32).ravel()
    out[0:50, 0] = v
    out[64:114, 0] = v
    return out


def kernel(x, edge_index, edge_weight, batch, num_graphs,
           W1, b1, W2, b2, W3, b3, W4, b4, gamma, beta,
           Wf1, bf1, Wf2, bf2, Wf3, bf3):
    assert int(num_graphs) == NUM_GRAPHS and x.shape == (N_NODES, 6)
    exp_batch = np.repeat(np.arange(NUM_GRAPHS), NPG)
    if not np.array_equal(np.asarray(batch), exp_batch):
        raise ValueError("non-contiguous batch layout not supported")

    A = _edge_matrices(edge_index, edge_weight)

    key = "prog"
    if key not in _cache:
        _cache[key] = build_program()
    nc, nb, gpad, nbh, gcols = _cache[key]

    f32 = lambda a: np.ascontiguousarray(np.asarray(a, np.float32))
    col = lambda a: f32(a).reshape(-1, 1)
    shared = dict(
        w1=f32(W1), w2=f32(W2), w3=f32(W3), w4=f32(W4),
        wf1=f32(Wf1), wf2=f32(Wf2), wf3=f32(Wf3),
        b1=col(b1), b2=col(b2), b3=col(b3),
        bf1=col(bf1), bf2=col(bf2), bf3=col(bf3),
        b4s=_stack2(b4), gammas=_stack2(gamma), betas=_stack2(beta),
    )
    in_maps = []
    for c in range(NCORES):
        xs = f32(x[c * GPC * NPG:(c + 1) * GPC * NPG])
        xin, ewin = _prep_core_inputs(xs, A[c * GPC:(c + 1) * GPC], nb, gpad)
        in_maps.append(dict(shared, xin=xin, ewin=ewin))

    global LAST_RESULTS
    LAST_RESULTS = run_bass_kernel_spmd(nc, in_maps, list(range(NCORES)),
                                        trace=TRACE)
    res = LAST_RESULTS.results

    # columns: even blocks first half, odd blocks second half
    nbh16 = nbh * 16
    g = np.arange(gpad)
    blk, gl = np.divmod(g, BG)
    colidx = np.where(blk % 2 == 0, (blk // 2) * 16 + gl,
                      nbh16 + (blk // 2) * 16 + gl)
    out = np.empty((NUM_GRAPHS, 2), np.float32)
    for c in range(NCORES):
        logits = np.asarray(res[c]["outp"])          # [2, gcols]
        out[c * GPC:(c + 1) * GPC] = logits[:, colidx[:GPC]].T
    return out
